# revision 1
# baseline (speedup 1.0000x reference)
"""CCConvLayer (GNN message passing) on 8 Trainium2 NeuronCores.

    x1  = x @ W.T                      # dense projection [N, 128]
    out = relu(segment_sum(x1[src] * vals[:, None], dst, N))

Strategy (edge/data parallel SpMM, dst-bucketed so no collective needed):
  * dst space is cut into 128-node blocks. Blocks are assigned to
    (core, slot) pairs, balanced by edge count. Every core owns the
    complete output rows for its blocks => no all-reduce; the host
    just re-assembles the slices.
  * Each core redundantly computes the dense projection x1 (cheap on PE)
    into a bf16 DRAM table.
  * Each core gathers x1 rows for its edges in bulk with dma_gather
    (one SDMA descriptor per edge). dma_gather indices are int16, so the
    table is addressed in two halves (rows < 32768 and >= 32768) and each
    slot's edges are split into a lo and a hi section.
  * Scatter-add per 128-edge tile: one-hot S[e, n] = (dst_local[e] == n)
    built by one DVE op, gathered rows scaled by vals on ScalarE, then
    PSUM += S.T @ (vals*G) on the tensor engine; ReLU on the way out.
  * SPMD: the instruction stream is identical on all cores; per-slot tile
    counts are shared across cores (max over the 8 blocks in the slot
    group), so only the DATA differs per core.
"""

import math

import numpy as np
import ml_dtypes

import concourse.bacc as bacc
import concourse.bass as bass
import concourse.mybir as mybir
import concourse.tile as tile
from concourse.bass_utils import run_bass_kernel_spmd

P = 128          # partitions / block size / edge-tile size
CH = 128         # in/out channels (problem-specific)
N_CORES = 8
MT = 512         # phase-1 node chunk
H_SPLIT = 32768  # int16 index limit for dma_gather
GC = 48          # gather chunk size in tiles (6144 edges / call)

F32 = mybir.dt.float32
BF16 = mybir.dt.bfloat16
I32 = mybir.dt.int32
I16 = mybir.dt.int16


def _wrap_idx(idx):
    """int16 index layout for dma_gather: element i at partition i%16,
    column i//16; 16-partition block replicated to all 128 partitions."""
    L = len(idx) // 16
    w = idx.reshape(L, 16).T.astype(np.int16)  # [16, L]
    return np.ascontiguousarray(np.tile(w, (8, 1)))  # [128, L]


def _plan_edges(src, dst, vals, n_nodes, n_cores, h_split):
    """Bucket edges by 128-node dst block, assign blocks to (slot, core),
    split each slot's edges into lo (src < h_split) / hi sections, pad each
    (slot, core, section) to T*128 edges shared across cores."""
    nb = math.ceil(n_nodes / P)
    nb_pad = math.ceil(nb / n_cores) * n_cores
    slots = nb_pad // n_cores

    blk = (dst // P).astype(np.int64)
    counts = np.bincount(blk, minlength=nb_pad)
    order = np.argsort(-counts, kind="stable")

    assign = np.empty((slots, n_cores), dtype=np.int64)
    totals = np.zeros(n_cores, dtype=np.int64)
    for s in range(slots):
        group = order[s * n_cores:(s + 1) * n_cores]  # desc counts
        cs = np.argsort(totals, kind="stable")  # least-loaded cores first
        for i, b in enumerate(group):
            assign[s, cs[i]] = b
            totals[cs[i]] += counts[b]

    # per-edge-per-core grouping
    eorder = np.argsort(blk, kind="stable")
    starts = np.zeros(nb_pad + 1, dtype=np.int64)
    np.cumsum(counts, out=starts[1:])

    # edge lists per (slot, core, section)
    lists = [[None] * n_cores for _ in range(slots)]
    TL = np.zeros(slots, dtype=np.int64)
    TH = np.zeros(slots, dtype=np.int64)
    for s in range(slots):
        for c in range(n_cores):
            b = int(assign[s, c])
            e = eorder[starts[b]:starts[b + 1]]
            lo = e[src[e] < h_split]
            hi = e[src[e] >= h_split]
            # sort by src: descriptors then walk the x1 table monotonically,
            # turning random 256B HBM reads into row-local ones
            lo = lo[np.argsort(src[lo], kind="stable")]
            hi = hi[np.argsort(src[hi], kind="stable")]
            lists[s][c] = (lo, hi)
            TL[s] = max(TL[s], -(-len(lo) // P))
            TH[s] = max(TH[s], -(-len(hi) // P))
        if TL[s] + TH[s] == 0:
            TL[s] = 1  # keep the psum chain non-empty
    KL = int(TL.sum())
    KH = int(TH.sum())
    K = KL + KH
    offL = np.zeros(slots + 1, dtype=np.int64)
    np.cumsum(TL, out=offL[1:])
    offH = np.zeros(slots + 1, dtype=np.int64)
    np.cumsum(TH, out=offH[1:])

    srcL = np.zeros((n_cores, KL * P), dtype=np.int64)
    srcH = np.zeros((n_cores, KH * P), dtype=np.int64)
    dstl_a = np.zeros((n_cores, K * P), dtype=np.float32)
    val_a = np.zeros((n_cores, K * P), dtype=np.float32)
    for s in range(slots):
        for c in range(n_cores):
            b = int(assign[s, c])
            lo, hi = lists[s][c]
            ll = int(offL[s]) * P
            srcL[c, ll:ll + len(lo)] = src[lo]
            dstl_a[c, ll:ll + len(lo)] = (dst[lo] - b * P).astype(np.float32)
            val_a[c, ll:ll + len(lo)] = vals[lo]
            ho = int(offH[s]) * P
            srcH[c, ho:ho + len(hi)] = src[hi] - h_split
            hh = (KL + int(offH[s])) * P
            dstl_a[c, hh:hh + len(hi)] = (dst[hi] - b * P).astype(np.float32)
            val_a[c, hh:hh + len(hi)] = vals[hi]

    idxL = np.stack([_wrap_idx(srcL[c]) for c in range(n_cores)]) \
        if KL else np.zeros((n_cores, P, 0), dtype=np.int16)
    idxH = np.stack([_wrap_idx(srcH[c]) for c in range(n_cores)]) \
        if KH else np.zeros((n_cores, P, 0), dtype=np.int16)

    # interleave dstl/vals: position j -> (tile j//P, partition j%P) => [P, K]
    dstl_i = np.ascontiguousarray(dstl_a.reshape(n_cores, K, P).transpose(0, 2, 1))
    val_i = np.ascontiguousarray(val_a.reshape(n_cores, K, P).transpose(0, 2, 1))
    meta = np.ascontiguousarray(
        np.concatenate([dstl_i, val_i], axis=2).astype(ml_dtypes.bfloat16))

    plan = {
        "assign": assign,
        "h": h_split,
        "slots": slots,
        "TL": TL.tolist(),
        "TH": TH.tolist(),
        "KL": KL,
        "KH": KH,
        "offL": offL.tolist(),
        "offH": offH.tolist(),
    }
    return plan, idxL, idxH, meta


def _build_nc(xrows, plan, n_cores, loop_n=1, mode="full"):
    """Build the SPMD Bass program (identical on every core).

    loop_n > 1 wraps the whole body in an on-device repeat loop; mode
    ("full" | "p1" | "p1g" | "nog") ablates phases — both used only by the
    timing harness."""
    slots = plan["slots"]
    TL, TH = plan["TL"], plan["TH"]
    KL, KH = plan["KL"], plan["KH"]
    offL, offH = plan["offL"], plan["offH"]
    K = KL + KH

    nc = bacc.Bacc(
        "TRN2",
        target_bir_lowering=False,
        debug=False,
        enable_asserts=False,
        num_devices=n_cores,
        num_swdge_queues=4,
    )
    # x uploaded pre-transposed [CH, xrows] so phase-1 loads are plain DMAs
    x_d = nc.dram_tensor("xt", [CH, xrows], BF16, kind="ExternalInput").ap()
    wt_d = nc.dram_tensor("wt", [CH, CH], BF16, kind="ExternalInput").ap()
    # dstl and vals packed side by side so one DMA (one semaphore) loads both
    # (bf16: dst_local ints <= 127 are exact; vals round same as in the S mul)
    mt_d = nc.dram_tensor("meta", [P, 2 * K], BF16, kind="ExternalInput").ap()
    il_d = (
        nc.dram_tensor("idxlo", [P, KL * 8], I16, kind="ExternalInput").ap()
        if KL else None
    )
    ih_d = (
        nc.dram_tensor("idxhi", [P, KH * 8], I16, kind="ExternalInput").ap()
        if KH else None
    )
    out_d = nc.dram_tensor("out", [slots * P, CH], F32, kind="ExternalOutput").ap()
    x1_d = nc.dram_tensor("x1", [xrows, CH], BF16).ap()

    with tile.TileContext(nc) as tc:
        if loop_n > 1:
            with tc.For_i(0, loop_n, 1):
                _emit_body(nc, tc, plan, xrows, x_d, wt_d, mt_d, il_d, ih_d,
                           out_d, x1_d, mode)
        else:
            _emit_body(nc, tc, plan, xrows, x_d, wt_d, mt_d, il_d, ih_d,
                       out_d, x1_d, mode)
    nc.compile()
    return nc


def _emit_body(nc, tc, plan, xrows, x_d, wt_d, mt_d, il_d, ih_d, out_d, x1_d,
               mode="full"):
    slots = plan["slots"]
    TL, TH = plan["TL"], plan["TH"]
    KL, KH = plan["KL"], plan["KH"]
    offL, offH = plan["offL"], plan["offH"]
    K = KL + KH
    if True:
        with (
            tc.tile_pool(name="const", bufs=1) as constp,
            tc.tile_pool(name="xt", bufs=3) as xtp,
            tc.tile_pool(name="stage", bufs=3) as stp,
            tc.tile_pool(name="gat", bufs=8) as gp,
            tc.tile_pool(name="sel", bufs=6) as selp,
            tc.tile_pool(name="res", bufs=2) as resp,
            tc.tile_pool(name="ps1", bufs=4, space="PSUM") as psp1,
            tc.tile_pool(name="ps2", bufs=4, space="PSUM") as psp2,
        ):
            wt_sb = constp.tile([CH, CH], BF16)
            nc.sync.dma_start(out=wt_sb[:], in_=wt_d[:])
            mt_sb = constp.tile([P, 2 * K], BF16)
            nc.sync.dma_start(out=mt_sb[:], in_=mt_d[:])
            dl_sb = mt_sb[:, :K]
            vl_sb = mt_sb[:, K:]
            if KL:
                il_sb = constp.tile([P, KL * 8], I16)
                nc.sync.dma_start(out=il_sb[:], in_=il_d[:])
            if KH:
                ih_sb = constp.tile([P, KH * 8], I16)
                nc.sync.dma_start(out=ih_sb[:], in_=ih_d[:])
            iota_i = constp.tile([P, P], I32)
            nc.gpsimd.iota(iota_i[:], pattern=[[1, P]], base=0, channel_multiplier=0)
            iota_f = constp.tile([P, P], BF16)
            nc.vector.tensor_copy(iota_f[:], iota_i[:])

            # ---- phase 1: x1 = x @ W.T, stored bf16 node-major in DRAM ----
            # hi-table rows (>= h) first so hi-section gathers overlap the
            # rest of phase 1
            h_chunk = min(plan["h"], xrows) // MT
            order = list(range(h_chunk, xrows // MT)) + list(range(h_chunk))
            for i in order:
                xt = xtp.tile([P, MT], BF16)
                nc.sync.dma_start(out=xt[:], in_=x_d[:, i * MT:(i + 1) * MT])
                stage = stp.tile([P, MT], BF16)
                for q in range(MT // P):
                    ps = psp1.tile([P, CH], F32)
                    nc.tensor.matmul(
                        out=ps[:],
                        lhsT=xt[:, q * P:(q + 1) * P],
                        rhs=wt_sb[:],
                        start=True,
                        stop=True,
                    )
                    nc.any.tensor_copy(out=stage[:, q * P:(q + 1) * P], in_=ps[:])
                nc.sync.dma_start(
                    out=x1_d[i * MT:(i + 1) * MT, :].rearrange(
                        "(q p) c -> p q c", p=P
                    ),
                    in_=stage[:].rearrange("p (q c) -> p q c", c=CH),
                )

            if mode == "p1":
                return
            # ---- phase 2: bulk gather + one-hot scatter matmuls ----
            # chunked dma_gather per section; chunk tiles issued lazily
            chunks = {}  # (sec, chunk_id) -> (tile, tiles_in_chunk)
            qrr = [0]  # round-robin SWDGE queue so desc-gen uses all 4 Q7 pairs

            def chunk_of(sec, t):
                cid = t // GC
                key = (sec, cid)
                if key not in chunks:
                    ksec = KL if sec == 0 else KH
                    nt = min(GC, ksec - cid * GC)
                    g = gp.tile([P, nt * CH], BF16, tag="gat")
                    isb = il_sb if sec == 0 else ih_sb
                    h = min(plan["h"], xrows)
                    table = x1_d[:h, :] if sec == 0 else x1_d[h:, :]
                    nc.gpsimd.dma_gather(
                        out_ap=g[:].rearrange("p (t c) -> p t c", c=CH),
                        in_ap=table,
                        idxs_ap=isb[:, cid * GC * 8:(cid * GC + nt) * 8],
                        num_idxs=nt * P,
                        num_idxs_reg=nt * P,
                        elem_size=CH,
                        single_packet=False,
                        queue_num=qrr[0],
                    )
                    qrr[0] = (qrr[0] + 1) % 4
                    if mode == "p1g":
                        # keep the gather alive with a tiny consumer
                        dummy = selp.tile([P, 1], F32, tag="dmy")
                        nc.vector.tensor_copy(dummy[:], g[:, :1])
                    chunks[key] = (g, nt)
                return chunks[key]

            for s in range(slots):
                # unified tile ids: lo tiles then hi tiles of this slot
                tiles = [(0, offL[s] + t) for t in range(TL[s])]
                tiles += [(1, offH[s] + t) for t in range(TH[s])]
                if mode == "p1g":
                    for sec, t in tiles:
                        chunk_of(sec, t)
                    continue
                ps = psp2.tile([P, CH], F32)
                for i, (sec, t) in enumerate(tiles):
                    u = t if sec == 0 else KL + t  # meta column index
                    if mode == "nog":
                        g = wt_sb
                    else:
                        g, _ = chunk_of(sec, t)
                    # S[e, n] = vals[e] * (dst_local[e] == n) in one DVE op
                    S = selp.tile([P, P], BF16, tag="sel")
                    nc.vector.scalar_tensor_tensor(
                        out=S[:],
                        in0=iota_f[:],
                        scalar=dl_sb[:, u:u + 1],
                        in1=vl_sb[:, u:u + 1].to_broadcast([P, P]),
                        op0=mybir.AluOpType.is_equal,
                        op1=mybir.AluOpType.mult,
                    )
                    nc.tensor.matmul(
                        out=ps[:],
                        lhsT=S[:],
                        rhs=(g[:] if mode == "nog"
                             else g[:, (t % GC) * CH:(t % GC + 1) * CH]),
                        start=(i == 0),
                        stop=(i == len(tiles) - 1),
                    )
                res = resp.tile([P, CH], F32)
                nc.scalar.activation(
                    out=res[:], in_=ps[:], func=mybir.ActivationFunctionType.Relu
                )
                nc.sync.dma_start(out=out_d[s * P:(s + 1) * P, :], in_=res[:])


_NC_CACHE = {}


def prepare(x, W, src, dst, vals, n_cores=N_CORES, h_split=H_SPLIT):
    """Host-side planning + input maps."""
    x = np.asarray(x, dtype=np.float32)
    W = np.asarray(W, dtype=np.float32)
    src = np.asarray(src).astype(np.int64)
    dst = np.asarray(dst).astype(np.int64)
    vals = np.asarray(vals, dtype=np.float32)

    n = x.shape[0]
    plan, idxL, idxH, meta = _plan_edges(src, dst, vals, n, n_cores, h_split)

    xrows = math.ceil(n / MT) * MT
    x_pad = np.zeros((CH, xrows), dtype=ml_dtypes.bfloat16)
    x_pad[:, :n] = x.astype(ml_dtypes.bfloat16).T
    wt = np.ascontiguousarray(W.T).astype(ml_dtypes.bfloat16)

    key = (xrows, n_cores, plan["h"], plan["KL"], plan["KH"],
           tuple(plan["TL"]), tuple(plan["TH"]))
    nc = _NC_CACHE.get(key)
    if nc is None:
        nc = _build_nc(xrows, plan, n_cores)
        _NC_CACHE[key] = nc

    in_maps = []
    for c in range(n_cores):
        m = {"xt": x_pad, "wt": wt, "meta": meta[c]}
        if plan["KL"]:
            m["idxlo"] = idxL[c]
        if plan["KH"]:
            m["idxhi"] = idxH[c]
        in_maps.append(m)
    return nc, in_maps, plan, n


def assemble(results, plan, n, n_cores=N_CORES):
    """Scatter per-core slot outputs back to the full [n, CH] output."""
    assign, slots = plan["assign"], plan["slots"]
    out_full = np.zeros((slots * n_cores * P, CH), dtype=np.float32)
    for c in range(n_cores):
        o = results[c]["out"]
        for s in range(slots):
            b = int(assign[s, c])
            out_full[b * P:(b + 1) * P] = o[s * P:(s + 1) * P]
    return out_full[:n]


def kernel(x, W, src, dst, vals, **_run_kwargs):
    nc, in_maps, plan, n = prepare(x, W, src, dst, vals)
    res = run_bass_kernel_spmd(
        nc, in_maps, core_ids=list(range(N_CORES)), **_run_kwargs
    )
    out = assemble(res.results, plan, n)
    if _run_kwargs:
        return out, res
    return out



# revision 6
# speedup vs baseline: 9.1887x; 9.1887x over previous
"""CCConvLayer (GNN message passing) on 8 Trainium2 NeuronCores.

    x1  = x @ W.T                      # dense projection [N, 128]
    out = relu(segment_sum(x1[src] * vals[:, None], dst, N))

Strategy (edge/data parallel SpMM, dst-bucketed so no collective needed):
  * dst space is cut into 128-node blocks. Blocks are assigned to
    (core, slot) pairs, balanced by edge count. Every core owns the
    complete output rows for its blocks => no all-reduce; the host
    just re-assembles the slices.
  * Each core redundantly computes the dense projection x1 (cheap on PE)
    into a bf16 DRAM table.
  * Each core gathers x1 rows for its edges in bulk with dma_gather
    (one SDMA descriptor per edge). dma_gather indices are int16, so the
    table is addressed in two halves (rows < 32768 and >= 32768) and each
    slot's edges are split into a lo and a hi section.
  * Scatter-add per 128-edge tile: one-hot S[e, n] = (dst_local[e] == n)
    built by one DVE op, gathered rows scaled by vals on ScalarE, then
    PSUM += S.T @ (vals*G) on the tensor engine; ReLU on the way out.
  * SPMD: the instruction stream is identical on all cores; per-slot tile
    counts are shared across cores (max over the 8 blocks in the slot
    group), so only the DATA differs per core.
"""

import math

import numpy as np
import ml_dtypes

import concourse.bacc as bacc
import concourse.bass as bass
import concourse.mybir as mybir
import concourse.tile as tile
from concourse.bass_utils import run_bass_kernel_spmd

P = 128          # partitions / block size / edge-tile size
CH = 128         # in/out channels (problem-specific)
N_CORES = 8
MT = 2048        # phase-1 node chunk
SC = 512         # x1 store-permutation granularity (nodes per store block)
H_SPLIT = 32768  # int16 index limit for dma_gather
GC = 48          # gather chunk size in tiles (6144 edges / call)

F32 = mybir.dt.float32
BF16 = mybir.dt.bfloat16
I32 = mybir.dt.int32
I16 = mybir.dt.int16


def _wrap_idx(idx):
    """int16 index layout for dma_gather: element i at partition i%16,
    column i//16; 16-partition block replicated to all 128 partitions."""
    L = len(idx) // 16
    w = idx.reshape(L, 16).T.astype(np.int16)  # [16, L]
    return np.ascontiguousarray(np.tile(w, (8, 1)))  # [128, L]


def _plan_edges(src, dst, vals, n_nodes, n_cores, h_split):
    """Bucket edges by 128-node dst block, assign blocks to (slot, core),
    split each slot's edges into lo (src < h_split) / hi sections, pad each
    (slot, core, section) to T*128 edges shared across cores."""
    nb = math.ceil(n_nodes / P)
    nb_pad = math.ceil(nb / n_cores) * n_cores
    slots = nb_pad // n_cores

    blk = (dst // P).astype(np.int64)
    counts = np.bincount(blk, minlength=nb_pad)
    order = np.argsort(-counts, kind="stable")

    assign = np.empty((slots, n_cores), dtype=np.int64)
    totals = np.zeros(n_cores, dtype=np.int64)
    for s in range(slots):
        group = order[s * n_cores:(s + 1) * n_cores]  # desc counts
        cs = np.argsort(totals, kind="stable")  # least-loaded cores first
        for i, b in enumerate(group):
            assign[s, cs[i]] = b
            totals[cs[i]] += counts[b]

    # per-edge-per-core grouping
    eorder = np.argsort(blk, kind="stable")
    starts = np.zeros(nb_pad + 1, dtype=np.int64)
    np.cumsum(counts, out=starts[1:])

    # edge lists per (slot, core, section)
    lists = [[None] * n_cores for _ in range(slots)]
    TL = np.zeros(slots, dtype=np.int64)
    TH = np.zeros(slots, dtype=np.int64)
    for s in range(slots):
        for c in range(n_cores):
            b = int(assign[s, c])
            e = eorder[starts[b]:starts[b + 1]]
            lo = e[src[e] < h_split]
            hi = e[src[e] >= h_split]
            # sort by src: descriptors then walk the x1 table monotonically,
            # turning random 256B HBM reads into row-local ones
            lo = lo[np.argsort(src[lo], kind="stable")]
            hi = hi[np.argsort(src[hi], kind="stable")]
            lists[s][c] = (lo, hi)
            TL[s] = max(TL[s], -(-len(lo) // P))
            TH[s] = max(TH[s], -(-len(hi) // P))
        if TL[s] + TH[s] == 0:
            TL[s] = 1  # keep the psum chain non-empty
    KL = int(TL.sum())
    KH = int(TH.sum())
    K = KL + KH
    offL = np.zeros(slots + 1, dtype=np.int64)
    np.cumsum(TL, out=offL[1:])
    offH = np.zeros(slots + 1, dtype=np.int64)
    np.cumsum(TH, out=offH[1:])

    srcL = np.zeros((n_cores, KL * P), dtype=np.int64)
    srcH = np.zeros((n_cores, KH * P), dtype=np.int64)
    dstl_a = np.zeros((n_cores, K * P), dtype=np.float32)
    val_a = np.zeros((n_cores, K * P), dtype=np.float32)
    for s in range(slots):
        for c in range(n_cores):
            b = int(assign[s, c])
            lo, hi = lists[s][c]
            ll = int(offL[s]) * P
            srcL[c, ll:ll + len(lo)] = src[lo]
            dstl_a[c, ll:ll + len(lo)] = (dst[lo] - b * P).astype(np.float32)
            val_a[c, ll:ll + len(lo)] = vals[lo]
            ho = int(offH[s]) * P
            srcH[c, ho:ho + len(hi)] = src[hi] - h_split
            hh = (KL + int(offH[s])) * P
            dstl_a[c, hh:hh + len(hi)] = (dst[hi] - b * P).astype(np.float32)
            val_a[c, hh:hh + len(hi)] = vals[hi]

    idxL = np.stack([_wrap_idx(srcL[c]) for c in range(n_cores)]) \
        if KL else np.zeros((n_cores, P, 0), dtype=np.int16)
    idxH = np.stack([_wrap_idx(srcH[c]) for c in range(n_cores)]) \
        if KH else np.zeros((n_cores, P, 0), dtype=np.int16)

    # interleave dstl/vals: position j -> (tile j//P, partition j%P) => [P, K]
    dstl_i = np.ascontiguousarray(dstl_a.reshape(n_cores, K, P).transpose(0, 2, 1))
    val_i = np.ascontiguousarray(val_a.reshape(n_cores, K, P).transpose(0, 2, 1))
    meta = np.ascontiguousarray(
        np.concatenate([dstl_i, val_i], axis=2).astype(ml_dtypes.bfloat16))

    plan = {
        "assign": assign,
        "h": h_split,
        "slots": slots,
        "TL": TL.tolist(),
        "TH": TH.tolist(),
        "KL": KL,
        "KH": KH,
        "offL": offL.tolist(),
        "offH": offH.tolist(),
    }
    return plan, idxL, idxH, meta


def _build_nc(xrows, plan, n_cores, loop_n=1, mode="full"):
    """Build the SPMD Bass program (identical on every core).

    loop_n > 1 wraps the whole body in an on-device repeat loop; mode
    ("full" | "p1" | "p1g" | "nog") ablates phases — both used only by the
    timing harness."""
    slots = plan["slots"]
    TL, TH = plan["TL"], plan["TH"]
    KL, KH = plan["KL"], plan["KH"]
    offL, offH = plan["offL"], plan["offH"]
    K = KL + KH

    nc = bacc.Bacc(
        "TRN2",
        target_bir_lowering=False,
        debug=False,
        enable_asserts=False,
        num_devices=n_cores,
        num_swdge_queues=4,
    )
    # x uploaded pre-transposed [CH, xrows] so phase-1 loads are plain DMAs
    x_d = nc.dram_tensor("xt", [CH, xrows], BF16, kind="ExternalInput").ap()
    wt_d = nc.dram_tensor("wt", [CH, CH], BF16, kind="ExternalInput").ap()
    # dstl and vals packed side by side so one DMA (one semaphore) loads both
    # (bf16: dst_local ints <= 127 are exact; vals round same as in the S mul)
    mt_d = nc.dram_tensor("meta", [P, 2 * K], BF16, kind="ExternalInput").ap()
    il_d = (
        nc.dram_tensor("idxlo", [P, KL * 8], I16, kind="ExternalInput").ap()
        if KL else None
    )
    ih_d = (
        nc.dram_tensor("idxhi", [P, KH * 8], I16, kind="ExternalInput").ap()
        if KH else None
    )
    out_d = nc.dram_tensor("out", [slots * P, CH], F32, kind="ExternalOutput").ap()
    x1_d = nc.dram_tensor("x1", [xrows, CH], BF16).ap()

    with tile.TileContext(nc) as tc:
        if loop_n > 1:
            with tc.For_i(0, loop_n, 1):
                _emit_body(nc, tc, plan, xrows, x_d, wt_d, mt_d, il_d, ih_d,
                           out_d, x1_d, mode)
        else:
            _emit_body(nc, tc, plan, xrows, x_d, wt_d, mt_d, il_d, ih_d,
                       out_d, x1_d, mode)
    nc.compile()
    return nc


def _emit_body(nc, tc, plan, xrows, x_d, wt_d, mt_d, il_d, ih_d, out_d, x1_d,
               mode="full"):
    slots = plan["slots"]
    TL, TH = plan["TL"], plan["TH"]
    KL, KH = plan["KL"], plan["KH"]
    offL, offH = plan["offL"], plan["offH"]
    K = KL + KH
    if True:
        with (
            tc.tile_pool(name="const", bufs=1) as constp,
            tc.tile_pool(name="xt", bufs=3) as xtp,
            tc.tile_pool(name="stage", bufs=3) as stp,
            tc.tile_pool(name="gat", bufs=8) as gp,
            tc.tile_pool(name="sel", bufs=6) as selp,
            tc.tile_pool(name="res", bufs=2) as resp,
            tc.tile_pool(name="ps1", bufs=4, space="PSUM") as psp1,
            tc.tile_pool(name="ps2", bufs=4, space="PSUM") as psp2,
        ):
            wt_sb = constp.tile([CH, CH], BF16)
            nc.sync.dma_start(out=wt_sb[:], in_=wt_d[:])
            mt_sb = constp.tile([P, 2 * K], BF16)
            nc.sync.dma_start(out=mt_sb[:], in_=mt_d[:])
            dl_sb = mt_sb[:, :K]
            vl_sb = mt_sb[:, K:]
            if KL:
                il_sb = constp.tile([P, KL * 8], I16)
                nc.sync.dma_start(out=il_sb[:], in_=il_d[:])
            if KH:
                ih_sb = constp.tile([P, KH * 8], I16)
                nc.sync.dma_start(out=ih_sb[:], in_=ih_d[:])
            iota_i = constp.tile([P, P], I32)
            nc.gpsimd.iota(iota_i[:], pattern=[[1, P]], base=0, channel_multiplier=0)
            iota_f = constp.tile([P, P], BF16)
            nc.vector.tensor_copy(iota_f[:], iota_i[:])

            # ---- phase 1: x1 = x @ W.T, stored bf16 in DRAM ----
            # Rows are chunk-permuted (node n -> row (n//MT)*MT + (n%P)*W +
            # (n%MT)//P, W=MT/P) so each partition's stage cols land as ONE
            # contiguous W*256B DRAM run instead of W scattered 256B rows.
            # Host permutes the gather indices to match, so this is free.
            # hi-table rows (>= h) first so hi-section gathers overlap the
            # rest of phase 1.
            W_ = MT // P
            h_chunk = min(plan["h"], xrows) // MT
            order = list(range(h_chunk, xrows // MT)) + list(range(h_chunk))
            for i in order:
                xt = xtp.tile([P, MT], BF16)
                nc.sync.dma_start(out=xt[:], in_=x_d[:, i * MT:(i + 1) * MT])
                stage = stp.tile([P, MT], BF16)
                for q4 in range(MT // 512):
                    ps = psp1.tile([P, 512], F32)
                    for j in range(4):
                        q = q4 * 4 + j
                        nc.tensor.matmul(
                            out=ps[:, j * P:(j + 1) * P],
                            lhsT=xt[:, q * P:(q + 1) * P],
                            rhs=wt_sb[:],
                            start=True,
                            stop=True,
                        )
                    nc.any.tensor_copy(
                        out=stage[:, q4 * 512:(q4 + 1) * 512], in_=ps[:]
                    )
                nc.sync.dma_start(
                    out=x1_d[i * MT:(i + 1) * MT, :].rearrange(
                        "(p w) c -> p w c", p=P
                    ),
                    in_=stage[:].rearrange("p (w c) -> p w c", c=CH),
                )

            if mode == "p1":
                return
            # ---- phase 2: bulk gather + one-hot scatter matmuls ----
            # chunked dma_gather per section; chunk tiles issued lazily
            chunks = {}  # (sec, chunk_id) -> (tile, tiles_in_chunk)
            qrr = [0]  # round-robin SWDGE queue so desc-gen uses all 4 Q7 pairs

            def chunk_of(sec, t):
                cid = t // GC
                key = (sec, cid)
                if key not in chunks:
                    ksec = KL if sec == 0 else KH
                    nt = min(GC, ksec - cid * GC)
                    g = gp.tile([P, nt * CH], BF16, tag="gat")
                    isb = il_sb if sec == 0 else ih_sb
                    h = min(plan["h"], xrows)
                    table = x1_d[:h, :] if sec == 0 else x1_d[h:, :]
                    nc.gpsimd.dma_gather(
                        out_ap=g[:].rearrange("p (t c) -> p t c", c=CH),
                        in_ap=table,
                        idxs_ap=isb[:, cid * GC * 8:(cid * GC + nt) * 8],
                        num_idxs=nt * P,
                        num_idxs_reg=nt * P,
                        elem_size=CH,
                        single_packet=False,
                        queue_num=qrr[0],
                    )
                    qrr[0] = (qrr[0] + 1) % 4
                    if mode == "p1g":
                        # keep the gather alive with a tiny consumer
                        dummy = selp.tile([P, 1], F32, tag="dmy")
                        nc.vector.tensor_copy(dummy[:], g[:, :1])
                    chunks[key] = (g, nt)
                return chunks[key]

            for s in range(slots):
                # unified tile ids: lo tiles then hi tiles of this slot
                tiles = [(0, offL[s] + t) for t in range(TL[s])]
                tiles += [(1, offH[s] + t) for t in range(TH[s])]
                if mode == "p1g":
                    for sec, t in tiles:
                        chunk_of(sec, t)
                    continue
                ps = psp2.tile([P, CH], F32)
                for i, (sec, t) in enumerate(tiles):
                    u = t if sec == 0 else KL + t  # meta column index
                    if mode == "nog":
                        g = wt_sb
                    else:
                        g, _ = chunk_of(sec, t)
                    # S[e, n] = vals[e] * (dst_local[e] == n) in one DVE op
                    S = selp.tile([P, P], BF16, tag="sel")
                    nc.vector.scalar_tensor_tensor(
                        out=S[:],
                        in0=iota_f[:],
                        scalar=dl_sb[:, u:u + 1],
                        in1=vl_sb[:, u:u + 1].to_broadcast([P, P]),
                        op0=mybir.AluOpType.is_equal,
                        op1=mybir.AluOpType.mult,
                    )
                    nc.tensor.matmul(
                        out=ps[:],
                        lhsT=S[:],
                        rhs=(g[:] if mode == "nog"
                             else g[:, (t % GC) * CH:(t % GC + 1) * CH]),
                        start=(i == 0),
                        stop=(i == len(tiles) - 1),
                    )
                res = resp.tile([P, CH], F32)
                nc.scalar.activation(
                    out=res[:], in_=ps[:], func=mybir.ActivationFunctionType.Relu
                )
                nc.sync.dma_start(out=out_d[s * P:(s + 1) * P, :], in_=res[:])


_NC_CACHE = {}


def prepare(x, W, src, dst, vals, n_cores=N_CORES, h_split=H_SPLIT):
    """Host-side planning + input maps."""
    x = np.asarray(x, dtype=np.float32)
    W = np.asarray(W, dtype=np.float32)
    src = np.asarray(src).astype(np.int64)
    dst = np.asarray(dst).astype(np.int64)
    vals = np.asarray(vals, dtype=np.float32)

    n = x.shape[0]
    # chunk-local store permutation (see phase-1 comment in _emit_body);
    # gather indices address the permuted table
    psrc = (src // MT) * MT + (src % P) * (MT // P) + (src % MT) // P
    plan, idxL, idxH, meta = _plan_edges(psrc, dst, vals, n, n_cores, h_split)

    xrows = math.ceil(n / MT) * MT
    x_pad = np.zeros((CH, xrows), dtype=ml_dtypes.bfloat16)
    x_pad[:, :n] = x.astype(ml_dtypes.bfloat16).T
    wt = np.ascontiguousarray(W.T).astype(ml_dtypes.bfloat16)

    key = (xrows, n_cores, plan["h"], plan["KL"], plan["KH"],
           tuple(plan["TL"]), tuple(plan["TH"]))
    nc = _NC_CACHE.get(key)
    if nc is None:
        nc = _build_nc(xrows, plan, n_cores)
        _NC_CACHE[key] = nc

    in_maps = []
    for c in range(n_cores):
        m = {"xt": x_pad, "wt": wt, "meta": meta[c]}
        if plan["KL"]:
            m["idxlo"] = idxL[c]
        if plan["KH"]:
            m["idxhi"] = idxH[c]
        in_maps.append(m)
    return nc, in_maps, plan, n


def assemble(results, plan, n, n_cores=N_CORES):
    """Scatter per-core slot outputs back to the full [n, CH] output."""
    assign, slots = plan["assign"], plan["slots"]
    out_full = np.zeros((slots * n_cores * P, CH), dtype=np.float32)
    for c in range(n_cores):
        o = results[c]["out"]
        for s in range(slots):
            b = int(assign[s, c])
            out_full[b * P:(b + 1) * P] = o[s * P:(s + 1) * P]
    return out_full[:n]


def kernel(x, W, src, dst, vals, **_run_kwargs):
    nc, in_maps, plan, n = prepare(x, W, src, dst, vals)
    res = run_bass_kernel_spmd(
        nc, in_maps, core_ids=list(range(N_CORES)), **_run_kwargs
    )
    out = assemble(res.results, plan, n)
    if _run_kwargs:
        return out, res
    return out

